# revision 1
# baseline (speedup 1.0000x reference)
"""Trainium2 Bass kernel for nn_PositionalEmbedding (embedding-lookup form).

Math: out[b, 2j]   = mean_k sin(params[k] * dc[b,k] * inv_freq[j])
      out[b, 2j+1] = mean_k cos(params[k] * dc[b,k] * inv_freq[j])

dc[b,k] are integers in [0, 60), so sin/cos over all (k, value) pairs form a
360-row lookup table T[k*60+v, 0:512] (sin/cos interleaved, pre-scaled 1/6)
that is built on-device from `params`.  The batch reduction then becomes, per
128-row tile, out_tile = onehotT.T @ T accumulated over 3 K-chunks of 120
dictionary rows, where onehotT[p, b] = (dc[b, k(p)] == v(p)) / 6 is built with
one small replication matmul + one fused DVE is_equal*scale per chunk.

Data parallel over 8 NeuronCores: each core handles 16384 rows.
"""

import numpy as np
import ml_dtypes

B = 131072
D = 512
NCOMP = 6
HYPER = 2100.0
NCORES = 8
BL = B // NCORES          # 16384 rows per core
P = 128                   # partitions / rows per output tile
NV = 60                   # dictionary values per component
ND = NCOMP * NV           # 360 dictionary rows
CK = 120                  # dictionary rows per K-chunk (2 components)
NCHUNK = ND // CK         # 3
GROUP = 4                 # output tiles per one-hot group (512 batch cols)

PI = float(np.pi)
TWO_PI = 2.0 * PI
# Mod-free range reduction (the DVE tensor-scalar ALU forbids MOD):
#   u = phase/(2*pi) + shift,  d = u - int_cast(u),  sin(2*pi*d - pi)
# equals sin(phase) for shift=0.5 and cos(phase) for shift=0.75, for ANY
# integer-rounding mode of the cast (trunc/floor/rne differ by a whole
# number, i.e. by 2*pi in the argument).
SHIFT_SIN = 0.5
SHIFT_COS = 0.75

_CACHE: dict = {}


def _host_constants():
    j = np.arange(0, D, 2, dtype=np.float32)
    inv_freq = np.float32(HYPER) ** (-(np.float32(2.0) * (j + np.float32(1.0))) / np.float32(D))
    # folded 1/(2*pi): the table build works on u = phase/(2*pi)
    scaled = (inv_freq.astype(np.float64) / (2.0 * np.pi)).astype(np.float32)
    invf2 = np.empty((D,), np.float32)
    invf2[0::2] = scaled
    invf2[1::2] = scaled
    invf2b = np.ascontiguousarray(np.broadcast_to(invf2, (CK, D)))

    # replication matrix: R[k, d] = 1 if k == d // NV
    repl = np.zeros((NCOMP, ND), np.float32)
    for k in range(NCOMP):
        repl[k, k * NV:(k + 1) * NV] = 1.0

    vvals = (np.arange(CK, dtype=np.float32) % NV).reshape(CK, 1)
    return invf2b, repl, vvals


def _build_nc(bl):
    import concourse.bacc as bacc
    import concourse.mybir as mybir
    from concourse import tile

    f32 = mybir.dt.float32
    f16 = mybir.dt.bfloat16
    Alu = mybir.AluOpType
    Act = mybir.ActivationFunctionType

    nc = bacc.Bacc(trn_type="TRN2")
    dct = nc.dram_tensor("dct", [NCOMP, bl], f16, kind="ExternalInput").ap()
    pvd = nc.dram_tensor("pvd", [CK, NCHUNK], f32, kind="ExternalInput").ap()
    r16 = nc.dram_tensor("r16", [NCOMP, ND], f16, kind="ExternalInput").ap()
    vvd = nc.dram_tensor("vvd", [CK, 1], f32, kind="ExternalInput").ap()
    ivd = nc.dram_tensor("ivd", [CK, D], f32, kind="ExternalInput").ap()
    out = nc.dram_tensor("out", [bl, D], f32, kind="ExternalOutput").ap()

    ntiles = bl // P
    ngroups = ntiles // GROUP

    with tile.TileContext(nc) as tc:
        with (
            tc.tile_pool(name="const", bufs=1) as cpool,
            tc.tile_pool(name="tbl", bufs=3) as wpool,
            tc.tile_pool(name="oh", bufs=9) as ohpool,
            tc.tile_pool(name="osb", bufs=6) as opool,
            tc.tile_pool(name="crep", bufs=3, space="PSUM") as ppool,
            tc.tile_pool(name="ops", bufs=4, space="PSUM") as qpool,
        ):
            # ---- constants into SBUF
            dct_sb = cpool.tile([NCOMP, bl], f16, tag="dct")
            nc.sync.dma_start(out=dct_sb[:, :], in_=dct)
            pv_sb = cpool.tile([CK, NCHUNK], f32, tag="pv")
            nc.sync.dma_start(out=pv_sb[:, :], in_=pvd)
            r16_sb = cpool.tile([NCOMP, ND], f16, tag="r16")
            nc.sync.dma_start(out=r16_sb[:, :], in_=r16)
            vv_sb = cpool.tile([CK, 1], f32, tag="vv")
            nc.sync.dma_start(out=vv_sb[:, :], in_=vvd)
            if_sb = cpool.tile([CK, D], f32, tag="if2")
            nc.sync.dma_start(out=if_sb[:, :], in_=ivd)
            mpi_sb = cpool.tile([CK, 1], f32, tag="mpi")
            nc.vector.memset(mpi_sb[:, :], -PI)

            # ---- main loop, software-pipelined EMISSION order.
            # Per group g:  main-matmuls(g) -> one-hot(g+1) -> copies+DMAs(g)
            # The is_equal of group g+1 waits on crep(g+1), which sits after
            # all of group g's matmuls in PE program order — so by the time
            # the PSUM->SBUF copies of group g run, DVE's view of the PE
            # clock already covers their matmuls and each copy needs only
            # its single ob-slot WAR (out-DMA) wait.  Every instruction
            # stays within walrus's one-sync-wait-per-instruction limit.
            def emit_onehot(g):
                ohs = []
                for c in range(NCHUNK):
                    crep = ppool.tile([CK, GROUP * P], f32, tag="crep")
                    nc.tensor.matmul(
                        crep[:, :], r16_sb[:, c * CK:(c + 1) * CK],
                        dct_sb[:, g * GROUP * P:(g + 1) * GROUP * P],
                        start=True, stop=True,
                    )
                    oh = ohpool.tile([CK, GROUP * P], f16, tag="oh")
                    nc.vector.tensor_scalar(
                        out=oh[:, :], in0=crep[:, :],
                        scalar1=vv_sb[:, :], scalar2=None,
                        op0=Alu.is_equal,
                    )
                    ohs.append(oh)
                return ohs

            ohs = emit_onehot(0)
            # ---- build sin/cos lookup table, 3 chunks of [120, 512] fp16
            shift_sb = cpool.tile([CK, D], f32, tag="shift")
            nc.vector.memset(shift_sb[:, 0::2], SHIFT_SIN)
            nc.vector.memset(shift_sb[:, 1::2], SHIFT_COS)
            tbl = []
            for c in range(NCHUNK):
                ph = wpool.tile([CK, D], f32, tag="ph")
                nc.vector.tensor_scalar_mul(ph[:, :], if_sb[:, :], pv_sb[:, c:c + 1])
                u = wpool.tile([CK, D], f32, tag="u")
                nc.vector.tensor_add(out=u[:, :], in0=ph[:, :], in1=shift_sb[:, :])
                ni = wpool.tile([CK, D], mybir.dt.int32, tag="ni")
                nc.vector.tensor_copy(out=ni[:, :], in_=u[:, :])
                nf = wpool.tile([CK, D], f32, tag="nf")
                nc.vector.tensor_copy(out=nf[:, :], in_=ni[:, :])
                d = wpool.tile([CK, D], f32, tag="d")
                nc.vector.tensor_sub(out=d[:, :], in0=u[:, :], in1=nf[:, :])
                # d in (-1,1) whatever rounding the cast used; wrap into
                # [0,1) so the Sin argument 2*pi*d - pi stays in [-pi, pi)
                mk = wpool.tile([CK, D], f32, tag="mk")
                nc.vector.tensor_scalar(
                    out=mk[:, :], in0=d[:, :], scalar1=0.0, scalar2=None,
                    op0=Alu.is_lt,
                )
                dw = wpool.tile([CK, D], f32, tag="dw")
                nc.vector.tensor_add(out=dw[:, :], in0=d[:, :], in1=mk[:, :])
                tt = cpool.tile([CK, D], f16, tag=f"tbl{c}")
                nc.scalar.activation(
                    tt[:, :], dw[:, :], Act.Sin, bias=mpi_sb[:, :], scale=TWO_PI
                )
                tbl.append(tt)

            for g in range(ngroups):
                pss = []
                for t in range(GROUP):
                    ps = qpool.tile([P, D], f32, tag="ops")
                    for c in range(NCHUNK):
                        nc.tensor.matmul(
                            ps[:, :], ohs[c][:, t * P:(t + 1) * P], tbl[c][:, :],
                            start=(c == 0), stop=(c == NCHUNK - 1),
                        )
                    pss.append(ps)
                if g + 1 < ngroups:
                    ohs = emit_onehot(g + 1)
                else:
                    # epilogue: advance DVE's PE clock past the last matmul
                    scrf = cpool.tile([P, 1], f32, tag="scrf")
                    nc.vector.tensor_copy(
                        out=scrf[0:1, :], in_=pss[GROUP - 1][0:1, 0:1]
                    )
                for t in range(GROUP):
                    ob = opool.tile([P, D], f32, tag="ob")
                    # 1/6 scale folded here so the one-hot stays an exact
                    # 1.0 in bf16 (halves the bf16 quantization error).
                    # t=0 on DVE (its PE wait is covered by is_eq(g+1) just
                    # before it in DVE program order), t=1..3 on ACT — keeps
                    # every copy engine under PE's ~3.2us/group so PE never
                    # micro-idles (HAM would throttle it to half rate).
                    if t == 0:
                        nc.vector.tensor_scalar_mul(ob[:, :], pss[t][:, :], 1.0 / NCOMP)
                    else:
                        nc.scalar.mul(ob[:, :], pss[t][:, :], 1.0 / NCOMP)
                    r0 = (g * GROUP + t) * P
                    nc.sync.dma_start(out=out[r0:r0 + P, :], in_=ob[:, :])

    # Bacc legalization: splits multi-sync-waits into EventSemaphores
    # (walrus allows at most one wait per instruction), allocates registers.
    nc.compile()
    return nc


def _get_nc(bl=BL):
    key = ("nc", bl)
    if key not in _CACHE:
        _CACHE[key] = _build_nc(bl)
    return _CACHE[key]


def _in_maps(date_components, params):
    dc = np.asarray(date_components).astype(np.int32, copy=False)
    prm = np.asarray(params).astype(np.float32, copy=False).reshape(NCOMP)
    invf2b, repl, vvals = _host_constants()
    r16 = repl.astype(ml_dtypes.bfloat16)
    # pv[p, c] = params[2c + p//60] * (p % 60), exactly the fp32 product the
    # reference forms (marshalling of the 6 params into the 360 dict rows)
    p_idx = np.arange(CK)
    pv = np.empty((CK, NCHUNK), np.float32)
    for c in range(NCHUNK):
        pv[:, c] = prm[2 * c + p_idx // NV] * (p_idx % NV).astype(np.float32)
    maps = []
    for i in range(NCORES):
        shard = dc[i * BL:(i + 1) * BL]
        dct = np.ascontiguousarray(shard.T).astype(ml_dtypes.bfloat16)
        maps.append({
            "dct": dct,
            "pvd": pv,
            "r16": r16,
            "vvd": vvals,
            "ivd": invf2b,
        })
    return maps


def kernel(date_components, params, _trace=False):
    from concourse.bass_utils import run_bass_kernel_spmd

    nc = _get_nc()
    maps = _in_maps(date_components, params)
    res = run_bass_kernel_spmd(
        nc, maps, core_ids=list(range(NCORES)),
        trace=_trace, trace_cores=[0] if _trace else None,
    )
    kernel.last_results = res
    return np.concatenate([r["out"] for r in res.results], axis=0)



# revision 4
# speedup vs baseline: 1.0243x; 1.0243x over previous
"""Trainium2 Bass kernel for nn_PositionalEmbedding (embedding-lookup form).

Math: out[b, 2j]   = mean_k sin(params[k] * dc[b,k] * inv_freq[j])
      out[b, 2j+1] = mean_k cos(params[k] * dc[b,k] * inv_freq[j])

dc[b,k] are integers in [0, 60), so sin/cos over all (k, value) pairs form a
360-row lookup table T[k*60+v, 0:512] (sin/cos interleaved, pre-scaled 1/6)
computed on the HOST from `params` and uploaded as bf16.  The batch reduction
becomes, per 128-row tile, out_tile = onehotT.T @ T accumulated over 3
K-chunks of 120 dictionary rows, where onehotT[p, b] = (dc[b, k(p)] == v(p))
is built with one small replication matmul (PE) + one is_equal (DVE) per
chunk per 512-column group.

Per group of 512 output rows (4 PSUM tiles): PSUM->SBUF copies go to
ACT (3) + GPSIMD (1) so DVE only runs is_equal, and the 4 tiles leave
through ONE dma_start (the sync engine's DGE config costs 565ns per
dma_start, so per-tile DMAs would make SP a co-bottleneck).  Batch rows
are pre-permuted on the host so DRAM row (4p + h) of a group maps to
stationary column p of tile-slot h: each DMA descriptor then covers 4
consecutive DRAM rows (8KB contiguous) from one SBUF partition.

Data parallel over 8 NeuronCores: each core handles 16384 rows.
"""

import numpy as np
import ml_dtypes

B = 131072
D = 512
NCOMP = 6
HYPER = 2100.0
NCORES = 8
BL = B // NCORES          # 16384 rows per core
P = 128                   # partitions / rows per output tile
NV = 60                   # dictionary values per component
ND = NCOMP * NV           # 360 dictionary rows
CK = 120                  # dictionary rows per K-chunk (2 components)
NCHUNK = ND // CK         # 3
GROUP = 4                 # output tiles per one-hot group (512 batch cols)

_CACHE: dict = {}


def _build_nc(bl):
    import concourse.bacc as bacc
    import concourse.mybir as mybir
    from concourse import tile

    f32 = mybir.dt.float32
    f16 = mybir.dt.bfloat16
    Alu = mybir.AluOpType

    nc = bacc.Bacc(trn_type="TRN2")
    dct = nc.dram_tensor("dct", [NCOMP, bl], f16, kind="ExternalInput").ap()
    r16 = nc.dram_tensor("r16", [NCOMP, ND], f16, kind="ExternalInput").ap()
    vvd = nc.dram_tensor("vvd", [CK, 1], f32, kind="ExternalInput").ap()
    tbd = nc.dram_tensor("tbd", [CK, NCHUNK * D], f16, kind="ExternalInput").ap()
    out = nc.dram_tensor("out", [bl, D], f32, kind="ExternalOutput").ap()

    ntiles = bl // P
    ngroups = ntiles // GROUP

    with tile.TileContext(nc) as tc:
        with (
            tc.tile_pool(name="const", bufs=1) as cpool,
            tc.tile_pool(name="oh", bufs=9) as ohpool,
            tc.tile_pool(name="osb", bufs=3) as opool,
            tc.tile_pool(name="crep", bufs=3, space="PSUM") as ppool,
            tc.tile_pool(name="ops", bufs=5, space="PSUM") as qpool,
        ):
            # ---- inputs into SBUF; split across DGE issuers so the SP
            # sequencer's 565ns-per-dma_start config doesn't serialize them.
            dct_sb = cpool.tile([NCOMP, bl], f16, tag="dct")
            nc.sync.dma_start(out=dct_sb[:, :], in_=dct)
            r16_sb = cpool.tile([NCOMP, ND], f16, tag="r16")
            nc.sync.dma_start(out=r16_sb[:, :], in_=r16)
            vv_sb = cpool.tile([CK, 1], f32, tag="vv")
            nc.sync.dma_start(out=vv_sb[:, :], in_=vvd)
            tb_sb = cpool.tile([CK, NCHUNK * D], f16, tag="tb")
            nc.scalar.dma_start(out=tb_sb[:, :], in_=tbd)

            def emit_onehot(g):
                ohs = []
                for c in range(NCHUNK):
                    crep = ppool.tile([CK, GROUP * P], f32, tag="crep")
                    nc.tensor.matmul(
                        crep[:, :], r16_sb[:, c * CK:(c + 1) * CK],
                        dct_sb[:, g * GROUP * P:(g + 1) * GROUP * P],
                        start=True, stop=True,
                    )
                    oh = ohpool.tile([CK, GROUP * P], f16, tag="oh")
                    nc.vector.tensor_scalar(
                        out=oh[:, :], in0=crep[:, :],
                        scalar1=vv_sb[:, :], scalar2=None,
                        op0=Alu.is_equal,
                    )
                    ohs.append(oh)
                return ohs

            ohs = emit_onehot(0)
            ohs_next = ohs
            for g in range(ngroups):
                ob = opool.tile([P, GROUP * D], f32, tag="ob")
                pss = []
                for t in range(GROUP):
                    ps = qpool.tile([P, D], f32, tag="ops")
                    for c in range(NCHUNK):
                        nc.tensor.matmul(
                            ps[:, :], ohs[c][:, t * P:(t + 1) * P],
                            tb_sb[:, c * D:(c + 1) * D],
                            start=(c == 0), stop=(c == NCHUNK - 1),
                        )
                    pss.append(ps)
                if g + 1 < ngroups:
                    ohs_next = emit_onehot(g + 1)
                for t in range(GROUP):
                    # GPSIMD can't read PSUM on TRN2; ACT does all 4 copies
                    # (~2.45us/group) and DVE keeps only is_equal (~2.1us),
                    # both under the ~2.8us/group DMA floor.
                    nc.scalar.copy(ob[:, t * D:(t + 1) * D], pss[t][:, :])
                # DRAM row (g*512 + 4p + h) <- ob[p, h*512:(h+1)*512]:
                # one dma_start per group, 8KB-contiguous descriptors.
                nc.sync.dma_start(
                    out=out[g * GROUP * P:(g + 1) * GROUP * P, :].rearrange(
                        "(p h) d -> p (h d)", h=GROUP),
                    in_=ob[:, :],
                )
                ohs = ohs_next

    # Bacc legalization: splits multi-sync-waits into EventSemaphores
    # (walrus allows at most one wait per instruction), allocates registers.
    nc.compile()
    return nc


def _get_nc(bl=BL):
    key = ("nc", bl)
    if key not in _CACHE:
        _CACHE[key] = _build_nc(bl)
    return _CACHE[key]


def _host_constants(prm):
    """Sin/cos lookup table (pre-scaled by 1/6), replication matrix, values."""
    j = np.arange(0, D, 2, dtype=np.float32)
    inv_freq = np.float32(HYPER) ** (
        -(np.float32(2.0) * (j + np.float32(1.0))) / np.float32(D))  # [256] f32
    p_idx = np.arange(CK)
    vals = (p_idx % NV).astype(np.float32)
    tb = np.empty((CK, NCHUNK * D), np.float32)
    inv6 = np.float32(1.0 / NCOMP)
    for c in range(NCHUNK):
        # pv = params[k(p)] * v(p) with the same f32 ops as the reference
        pv = prm[2 * c + p_idx // NV] * vals                      # [120] f32
        phase = pv[:, None] * inv_freq[None, :]                   # [120, 256] f32
        tb[:, c * D + 0:c * D + D:2] = np.sin(phase) * inv6
        tb[:, c * D + 1:c * D + D:2] = np.cos(phase) * inv6
    tb16 = tb.astype(ml_dtypes.bfloat16)

    repl = np.zeros((NCOMP, ND), np.float32)
    for k in range(NCOMP):
        repl[k, k * NV:(k + 1) * NV] = 1.0
    r16 = repl.astype(ml_dtypes.bfloat16)
    vv = vals.reshape(CK, 1)
    return tb16, r16, vv


def _in_maps(date_components, params):
    dc = np.asarray(date_components).astype(np.int32, copy=False)
    prm = np.asarray(params).astype(np.float32, copy=False).reshape(NCOMP)
    tb16, r16, vv = _host_constants(prm)

    # batch permutation: stationary column p of tile-slot h in group g holds
    # original row g*512 + 4p + h, so the group's single out-DMA writes DRAM
    # rows in natural order with 4-row-contiguous descriptors.
    jj = np.arange(GROUP * P)
    src = GROUP * (jj % P) + (jj // P)
    perm = (np.arange(0, BL, GROUP * P)[:, None] + src[None, :]).ravel()

    maps = []
    for i in range(NCORES):
        shard = dc[i * BL:(i + 1) * BL]
        dctm = np.ascontiguousarray(shard[perm].T).astype(ml_dtypes.bfloat16)
        maps.append({
            "dct": dctm,
            "r16": r16,
            "vvd": vv,
            "tbd": tb16,
        })
    return maps


def kernel(date_components, params, _trace=False):
    from concourse.bass_utils import run_bass_kernel_spmd

    nc = _get_nc()
    maps = _in_maps(date_components, params)
    res = run_bass_kernel_spmd(
        nc, maps, core_ids=list(range(NCORES)),
        trace=_trace, trace_cores=[0] if _trace else None,
    )
    kernel.last_results = res
    return np.concatenate([r["out"] for r in res.results], axis=0)


# revision 6
# speedup vs baseline: 1.0830x; 1.0574x over previous
"""Trainium2 Bass kernel for nn_PositionalEmbedding (embedding-lookup form).

Math: out[b, 2j]   = mean_k sin(params[k] * dc[b,k] * inv_freq[j])
      out[b, 2j+1] = mean_k cos(params[k] * dc[b,k] * inv_freq[j])

dc[b,k] are integers in [0, 60), so sin/cos over all (k, value) pairs form a
360-row lookup table T[k*60+v, 0:512] (sin/cos interleaved, pre-scaled 1/6)
computed on the HOST from `params` and uploaded as bf16.  The batch reduction
becomes, per 128-row tile, out_tile = onehotT.T @ T accumulated over 3
K-chunks of 120 dictionary rows, where onehotT[p, b] = (dc[b, k(p)] == v(p))
is built with one small replication matmul (PE) + one is_equal (DVE) per
chunk per 512-column group.

Per group of 512 output rows (4 PSUM tiles): PSUM->SBUF copies go to
ACT (3) + GPSIMD (1) so DVE only runs is_equal, and the 4 tiles leave
through ONE dma_start (the sync engine's DGE config costs 565ns per
dma_start, so per-tile DMAs would make SP a co-bottleneck).  Batch rows
are pre-permuted on the host so DRAM row (4p + h) of a group maps to
stationary column p of tile-slot h: each DMA descriptor then covers 4
consecutive DRAM rows (8KB contiguous) from one SBUF partition.

Data parallel over 8 NeuronCores: each core handles 16384 rows.
"""

import numpy as np
import ml_dtypes

B = 131072
D = 512
NCOMP = 6
HYPER = 2100.0
NCORES = 8
BL = B // NCORES          # 16384 rows per core
P = 128                   # partitions / rows per output tile
NV = 60                   # dictionary values per component
ND = NCOMP * NV           # 360 dictionary rows
CK = 120                  # dictionary rows per K-chunk (2 components)
NCHUNK = ND // CK         # 3
GROUP = 4                 # output tiles per one-hot group (512 batch cols)

_CACHE: dict = {}


def _build_nc(bl):
    import concourse.bacc as bacc
    import concourse.mybir as mybir
    from concourse import tile

    f32 = mybir.dt.float32
    f16 = mybir.dt.bfloat16
    Alu = mybir.AluOpType

    nc = bacc.Bacc(trn_type="TRN2")
    dct = nc.dram_tensor("dct", [NCOMP, bl], f16, kind="ExternalInput").ap()
    r16 = nc.dram_tensor("r16", [NCOMP, ND], f16, kind="ExternalInput").ap()
    vvd = nc.dram_tensor("vvd", [CK, 1], f32, kind="ExternalInput").ap()
    tbd = nc.dram_tensor("tbd", [CK, NCHUNK * D], f16, kind="ExternalInput").ap()
    out = nc.dram_tensor("out", [bl, D], f32, kind="ExternalOutput").ap()

    ntiles = bl // P
    ngroups = ntiles // GROUP

    with tile.TileContext(nc) as tc:
        with (
            tc.tile_pool(name="const", bufs=1) as cpool,
            tc.tile_pool(name="oh", bufs=9) as ohpool,
            tc.tile_pool(name="osb", bufs=3) as opool,
            tc.tile_pool(name="crep", bufs=3, space="PSUM") as ppool,
            tc.tile_pool(name="ops", bufs=5, space="PSUM") as qpool,
        ):
            # ---- inputs into SBUF; split across DGE issuers so the SP
            # sequencer's 565ns-per-dma_start config doesn't serialize them.
            # dct is 32KB on each of 6 partitions: without max_dma_last_dim
            # it lowers to 6 whole-row descriptors and a single descriptor
            # only streams at ~5.5 B/ns — split into 2KB descriptors so all
            # 16 DMA queues pipeline it (~25 B/ns each).
            dct_sb = cpool.tile([NCOMP, bl], f16, tag="dct")
            nc.sync.dma_start(out=dct_sb[:, :], in_=dct, max_dma_last_dim=1024)
            r16_sb = cpool.tile([NCOMP, ND], f16, tag="r16")
            nc.sync.dma_start(out=r16_sb[:, :], in_=r16)
            vv_sb = cpool.tile([CK, 1], f32, tag="vv")
            nc.sync.dma_start(out=vv_sb[:, :], in_=vvd)
            tb_sb = cpool.tile([CK, NCHUNK * D], f16, tag="tb")
            nc.scalar.dma_start(out=tb_sb[:, :], in_=tbd)

            def emit_onehot(g):
                ohs = []
                for c in range(NCHUNK):
                    crep = ppool.tile([CK, GROUP * P], f32, tag="crep")
                    nc.tensor.matmul(
                        crep[:, :], r16_sb[:, c * CK:(c + 1) * CK],
                        dct_sb[:, g * GROUP * P:(g + 1) * GROUP * P],
                        start=True, stop=True,
                    )
                    oh = ohpool.tile([CK, GROUP * P], f16, tag="oh")
                    nc.vector.tensor_scalar(
                        out=oh[:, :], in0=crep[:, :],
                        scalar1=vv_sb[:, :], scalar2=None,
                        op0=Alu.is_equal,
                    )
                    ohs.append(oh)
                return ohs

            # one-hot generation runs TWO groups ahead: the 3 serial
            # is_equal ops (~2.1us on DVE) for group g+2 overlap the main
            # matmuls of g/g+1 instead of sitting inside the PE->DVE->PE
            # critical cycle (which cost ~1.3us/group of DMA idle).
            oh_q = [emit_onehot(0), emit_onehot(1)]
            for g in range(ngroups):
                ohs = oh_q.pop(0)
                ob = opool.tile([P, GROUP * D], f32, tag="ob")
                pss = []
                for t in range(GROUP):
                    ps = qpool.tile([P, D], f32, tag="ops")
                    for c in range(NCHUNK):
                        nc.tensor.matmul(
                            ps[:, :], ohs[c][:, t * P:(t + 1) * P],
                            tb_sb[:, c * D:(c + 1) * D],
                            start=(c == 0), stop=(c == NCHUNK - 1),
                        )
                    pss.append(ps)
                if g + 2 < ngroups:
                    oh_q.append(emit_onehot(g + 2))
                for t in range(GROUP):
                    # GPSIMD can't read PSUM on TRN2; ACT does all 4 copies
                    # (~2.45us/group) and DVE keeps only is_equal (~2.1us),
                    # both under the ~2.8us/group DMA floor.
                    nc.scalar.copy(ob[:, t * D:(t + 1) * D], pss[t][:, :])
                # DRAM row (g*512 + 4p + h) <- ob[p, h*512:(h+1)*512]:
                # one dma_start per group, 8KB-contiguous descriptors.
                nc.sync.dma_start(
                    out=out[g * GROUP * P:(g + 1) * GROUP * P, :].rearrange(
                        "(p h) d -> p (h d)", h=GROUP),
                    in_=ob[:, :],
                )

    # Bacc legalization: splits multi-sync-waits into EventSemaphores
    # (walrus allows at most one wait per instruction), allocates registers.
    nc.compile()
    return nc


def _get_nc(bl=BL):
    key = ("nc", bl)
    if key not in _CACHE:
        _CACHE[key] = _build_nc(bl)
    return _CACHE[key]


def _host_constants(prm):
    """Sin/cos lookup table (pre-scaled by 1/6), replication matrix, values."""
    j = np.arange(0, D, 2, dtype=np.float32)
    inv_freq = np.float32(HYPER) ** (
        -(np.float32(2.0) * (j + np.float32(1.0))) / np.float32(D))  # [256] f32
    p_idx = np.arange(CK)
    vals = (p_idx % NV).astype(np.float32)
    tb = np.empty((CK, NCHUNK * D), np.float32)
    inv6 = np.float32(1.0 / NCOMP)
    for c in range(NCHUNK):
        # pv = params[k(p)] * v(p) with the same f32 ops as the reference
        pv = prm[2 * c + p_idx // NV] * vals                      # [120] f32
        phase = pv[:, None] * inv_freq[None, :]                   # [120, 256] f32
        tb[:, c * D + 0:c * D + D:2] = np.sin(phase) * inv6
        tb[:, c * D + 1:c * D + D:2] = np.cos(phase) * inv6
    tb16 = tb.astype(ml_dtypes.bfloat16)

    repl = np.zeros((NCOMP, ND), np.float32)
    for k in range(NCOMP):
        repl[k, k * NV:(k + 1) * NV] = 1.0
    r16 = repl.astype(ml_dtypes.bfloat16)
    vv = vals.reshape(CK, 1)
    return tb16, r16, vv


def _in_maps(date_components, params):
    dc = np.asarray(date_components).astype(np.int32, copy=False)
    prm = np.asarray(params).astype(np.float32, copy=False).reshape(NCOMP)
    tb16, r16, vv = _host_constants(prm)

    # batch permutation: stationary column p of tile-slot h in group g holds
    # original row g*512 + 4p + h, so the group's single out-DMA writes DRAM
    # rows in natural order with 4-row-contiguous descriptors.
    jj = np.arange(GROUP * P)
    src = GROUP * (jj % P) + (jj // P)
    perm = (np.arange(0, BL, GROUP * P)[:, None] + src[None, :]).ravel()

    maps = []
    for i in range(NCORES):
        shard = dc[i * BL:(i + 1) * BL]
        dctm = np.ascontiguousarray(shard[perm].T).astype(ml_dtypes.bfloat16)
        maps.append({
            "dct": dctm,
            "r16": r16,
            "vvd": vv,
            "tbd": tb16,
        })
    return maps


def kernel(date_components, params, _trace=False):
    from concourse.bass_utils import run_bass_kernel_spmd

    nc = _get_nc()
    maps = _in_maps(date_components, params)
    res = run_bass_kernel_spmd(
        nc, maps, core_ids=list(range(NCORES)),
        trace=_trace, trace_cores=[0] if _trace else None,
    )
    kernel.last_results = res
    return np.concatenate([r["out"] for r in res.results], axis=0)


# revision 13
# speedup vs baseline: 1.2656x; 1.1685x over previous
"""Trainium2 Bass kernel for nn_PositionalEmbedding (embedding-lookup form).

Math: out[b, 2j]   = mean_k sin(params[k] * dc[b,k] * inv_freq[j])
      out[b, 2j+1] = mean_k cos(params[k] * dc[b,k] * inv_freq[j])

dc[b,k] are integers in [0, 60), so sin/cos over all (k, value) pairs form a
360-row lookup table (pre-scaled 1/6, bf16) computed on the HOST from
`params`.  The batch reduction becomes, per 128-row tile,
out_tile = onehotT.T @ T accumulated over 3 K-chunks of 120 dictionary rows.

The PE sequencer issues ~4.6M instr/s (one matmul per ~216ns regardless of
size), so instruction COUNT per 512-row group is the scarce resource.  The
dictionary is laid out so chunk c, partition p holds (component p%6, value
20c + p//6): the replicated components crep[p,b] = dc[b, p%6] are then the
SAME for all 3 chunks -> ONE replication matmul per group (instead of 3)
feeding 3 is_equal ops against per-chunk value columns.  13 PE instructions
per group (~2.8us) ~= the 2.8us/group DMA floor for the fp32 output.

Per group of 512 output rows (4 PSUM tiles): PSUM->SBUF copies go to ACT
(GPSIMD cannot access PSUM on TRN2), the one-hot pipeline runs two groups
ahead so DVE's 3 serial is_equal overlap main matmuls, and the 4 tiles
leave through ONE dma_start (SP's DGE config costs 565ns per dma_start).
Batch rows are pre-permuted on the host so DRAM row (4p + h) of a group
maps to stationary column p of tile-slot h: each DMA descriptor then
covers 4 consecutive DRAM rows (8KB contiguous) from one SBUF partition.

dct is uploaded as [128, 4096] (8KB/partition; a [6, 16384] layout would
bottleneck on the ~5.4 B/ns per-partition SBUF write port for ~6us), with
component rows at base partitions {0,32,64,96} because matmul tile
positions must be multiples of 32.

Data parallel over 8 NeuronCores: each core handles 16384 rows.
"""

import numpy as np
import ml_dtypes

B = 131072
D = 512
NCOMP = 6
HYPER = 2100.0
NCORES = 8
BL = B // NCORES          # 16384 rows per core
P = 128                   # partitions / rows per output tile
NV = 60                   # dictionary values per component
ND = NCOMP * NV           # 360 dictionary rows
CK = 120                  # dictionary rows per K-chunk
NCHUNK = ND // CK         # 3
NVC = NV // NCHUNK        # 20 values per component per chunk
GROUP = 4                 # output tiles per one-hot group (512 batch cols)
# dct partition-blocks: matmul operands may only start at base partition
# 0/32/64 (bass_rust lowering limit), so the 32 groups split 11/11/10
# across three blocks to bound the per-partition SBUF write-port time.
DCTB = 3
GSPLIT = (0, 11, 22, 32)  # group ranges per block

_CACHE: dict = {}


def _build_nc(bl):
    import concourse.bacc as bacc
    import concourse.mybir as mybir
    from concourse import tile

    f32 = mybir.dt.float32
    f16 = mybir.dt.bfloat16
    Alu = mybir.AluOpType

    ntiles = bl // P
    ngroups = ntiles // GROUP             # 32
    colb = (GSPLIT[1] - GSPLIT[0]) * GROUP * P   # dct cols in widest block

    nc = bacc.Bacc(trn_type="TRN2")
    dct = nc.dram_tensor("dct", [64 + NCOMP, colb], f16, kind="ExternalInput").ap()
    r6d = nc.dram_tensor("r6d", [64 + NCOMP, CK], f16, kind="ExternalInput").ap()
    vvd = nc.dram_tensor("vvd", [CK, NCHUNK], f32, kind="ExternalInput").ap()
    tbd = nc.dram_tensor("tbd", [CK, NCHUNK * D], f16, kind="ExternalInput").ap()
    out = nc.dram_tensor("out", [bl, D], f32, kind="ExternalOutput").ap()

    with tile.TileContext(nc) as tc:
        with (
            tc.tile_pool(name="const", bufs=1) as cpool,
            tc.tile_pool(name="oh", bufs=3) as ohpool,
            tc.tile_pool(name="osb", bufs=4) as opool,
            tc.tile_pool(name="crep", bufs=2, space="PSUM") as ppool,
            tc.tile_pool(name="ops", bufs=6, space="PSUM") as qpool,
        ):
            dct_sb = cpool.tile([64 + NCOMP, colb], f16, tag="dct")
            nc.sync.dma_start(out=dct_sb[:, :], in_=dct)
            r6_sb = cpool.tile([64 + NCOMP, CK], f16, tag="r6")
            nc.sync.dma_start(out=r6_sb[:, :], in_=r6d)
            vv_sb = cpool.tile([CK, NCHUNK], f32, tag="vv")
            nc.sync.dma_start(out=vv_sb[:, :], in_=vvd)
            tb_sb = cpool.tile([CK, NCHUNK * D], f16, tag="tb")
            nc.scalar.dma_start(out=tb_sb[:, :], in_=tbd)

            def emit_onehot(g):
                j = 0 if g < GSPLIT[1] else (1 if g < GSPLIT[2] else 2)
                c0 = (g - GSPLIT[j]) * GROUP * P
                crep = ppool.tile([CK, GROUP * P], f32, tag="crep")
                nc.tensor.matmul(
                    crep[:, :], r6_sb[32 * j:32 * j + NCOMP, :],
                    dct_sb[32 * j:32 * j + NCOMP, c0:c0 + GROUP * P],
                    start=True, stop=True,
                )
                oh = ohpool.tile([CK, NCHUNK * GROUP * P], f16, tag="oh")
                for c in range(NCHUNK):
                    nc.vector.tensor_scalar(
                        out=oh[:, c * GROUP * P:(c + 1) * GROUP * P],
                        in0=crep[:, :],
                        scalar1=vv_sb[:, c:c + 1], scalar2=None,
                        op0=Alu.is_equal,
                    )
                return oh

            oh_q = [emit_onehot(0), emit_onehot(1)]
            for g in range(ngroups):
                oh = oh_q.pop(0)
                ob = opool.tile([P, GROUP * D], f32, tag="ob")
                pss = []
                for t in range(GROUP):
                    ps = qpool.tile([P, D], f32, tag="ops")
                    for c in range(NCHUNK):
                        nc.tensor.matmul(
                            ps[:, :],
                            oh[:, c * GROUP * P + t * P:c * GROUP * P + (t + 1) * P],
                            tb_sb[:, c * D:(c + 1) * D],
                            start=(c == 0), stop=(c == NCHUNK - 1),
                        )
                    pss.append(ps)
                if g + 2 < ngroups:
                    oh_q.append(emit_onehot(g + 2))
                for t in range(GROUP):
                    nc.scalar.copy(ob[:, t * D:(t + 1) * D], pss[t][:, :])
                nc.sync.dma_start(
                    out=out[g * GROUP * P:(g + 1) * GROUP * P, :].rearrange(
                        "(p h) d -> p (h d)", h=GROUP),
                    in_=ob[:, :],
                )

    # Bacc legalization: splits multi-sync-waits into EventSemaphores
    # (walrus allows at most one wait per instruction), allocates registers.
    nc.compile()
    return nc


def _get_nc(bl=BL):
    key = ("nc", bl)
    if key not in _CACHE:
        _CACHE[key] = _build_nc(bl)
    return _CACHE[key]


def _host_constants(prm):
    """Lookup table (pre-scaled 1/6), replication matrices, value columns.

    Dictionary layout: chunk c, partition p <-> (component p%6, value
    20c + p//6).
    """
    j = np.arange(0, D, 2, dtype=np.float32)
    inv_freq = np.float32(HYPER) ** (
        -(np.float32(2.0) * (j + np.float32(1.0))) / np.float32(D))  # [256] f32
    p_idx = np.arange(CK)
    kk = p_idx % NCOMP                     # component per partition
    inv6 = np.float32(1.0 / NCOMP)
    tb = np.empty((CK, NCHUNK * D), np.float32)
    vv = np.empty((CK, NCHUNK), np.float32)
    for c in range(NCHUNK):
        vals = (NVC * c + p_idx // NCOMP).astype(np.float32)
        vv[:, c] = vals
        pv = prm[kk] * vals                                   # [120] f32
        phase = pv[:, None] * inv_freq[None, :]               # [120, 256] f32
        tb[:, c * D + 0:c * D + D:2] = np.sin(phase) * inv6
        tb[:, c * D + 1:c * D + D:2] = np.cos(phase) * inv6
    tb16 = tb.astype(ml_dtypes.bfloat16)

    # replication matrix at each dct base partition: r6[32j + k, p] = (p%6==k)
    r6 = np.zeros((64 + NCOMP, CK), np.float32)
    for jb in range(DCTB):
        for k in range(NCOMP):
            r6[32 * jb + k, kk == k] = 1.0
    r6 = r6.astype(ml_dtypes.bfloat16)
    return tb16, r6, vv


def _in_maps(date_components, params):
    dc = np.asarray(date_components).astype(np.int32, copy=False)
    prm = np.asarray(params).astype(np.float32, copy=False).reshape(NCOMP)
    tb16, r6, vv = _host_constants(prm)

    # batch permutation: stationary column p of tile-slot h in group g holds
    # original row g*512 + 4p + h, so the group's single out-DMA writes DRAM
    # rows in natural order with 4-row-contiguous descriptors.
    jj = np.arange(GROUP * P)
    src = GROUP * (jj % P) + (jj // P)
    perm = (np.arange(0, BL, GROUP * P)[:, None] + src[None, :]).ravel()

    colb = (GSPLIT[1] - GSPLIT[0]) * GROUP * P
    maps = []
    for i in range(NCORES):
        shard = dc[i * BL:(i + 1) * BL]
        dctt = np.ascontiguousarray(shard[perm].T)            # [6, BL]
        # [70, 5632]: col-block j of the 6 component rows at partitions 32j+
        dctm = np.zeros((64 + NCOMP, colb), np.float32)
        for jb in range(DCTB):
            lo = GSPLIT[jb] * GROUP * P
            hi = GSPLIT[jb + 1] * GROUP * P
            dctm[32 * jb:32 * jb + NCOMP, 0:hi - lo] = dctt[:, lo:hi]
        maps.append({
            "dct": dctm.astype(ml_dtypes.bfloat16),
            "r6d": r6,
            "vvd": vv,
            "tbd": tb16,
        })
    return maps


def kernel(date_components, params, _trace=False):
    from concourse.bass_utils import run_bass_kernel_spmd

    nc = _get_nc()
    maps = _in_maps(date_components, params)
    res = run_bass_kernel_spmd(
        nc, maps, core_ids=list(range(NCORES)),
        trace=_trace, trace_cores=[0] if _trace else None,
    )
    kernel.last_results = res
    return np.concatenate([r["out"] for r in res.results], axis=0)
